# revision 19
# baseline (speedup 1.0000x reference)
"""Trainium2 Bass kernel for DeformationNetworkGraphConvolutionalFullRes.

Full (unsharded) inputs in, full output out. Data-parallel over the 4 meshes:
core m processes mesh m (cores 4-7 idle). Inside each core:

  - vert_align sampling is computed as (S @ F) @ W == S @ (F @ W): per feature
    map, F[C,HW] @ Wslice[C,128] -> G[HW,128] (tiny matmuls), then the sparse
    bilinear operator S (4 nonzeros/row) is applied as dense [128px, 512vert]
    blocks (built host-side from the vertex coordinates) streamed into the
    TensorEngine, accumulating over maps/pixel-tiles in PSUM. Vertices are
    pre-sorted by image cell so each 512-vertex chunk touches few pixel tiles.
  - Each GraphConv layer: h1 = x@W1 rows are written to HBM; messages
    h1[src] are pulled with dma_gather in dst-sorted edge order; the
    segmented sum over edges is done as one-hot matmuls (one-hots built
    on-device with a bulk is_equal) accumulating in PSUM on top of
    h0 = x@W0 (+ rank-1 image-encoding term), then ReLU writes the
    transposed activations for the next layer directly.
"""

import numpy as np
from contextlib import ExitStack

import concourse.bass as bass
import concourse.tile as tile
from concourse import bacc, mybir
from concourse.bass_utils import run_bass_kernel_spmd

# ---------------- problem constants (hardcoded per spec) ----------------
B = 4
V = 10242
E_PER = 30720
HID = 128
MAPS = [(256, 56), (512, 28), (1024, 14), (2048, 7)]  # (C, H==W)
CH_OFF = [0, 256, 768, 1792, 3840]

VP = 10752            # padded vertex count: 21 chunks of 512 = 84 tiles of 128
NT = VP // 128        # 84 vertex tiles
NVCH = VP // 512      # 21 vertex chunks (sampling)
NSUB = 7              # edge subchunks (of 128) per dst tile (padded)
GT = 2                # dst tiles per gather group
NGRP = NT // GT       # 21 gather groups
SUB_G = GT * NSUB     # subchunks per group
TOT_SUB = NT * NSUB   # total subchunks per mesh

F32 = mybir.dt.float32
I32 = mybir.dt.int32
I16 = mybir.dt.int16
AF = mybir.ActivationFunctionType


def _corners(grid, W):
    """grid [V,2] in [-1,1] -> list of (pix_idx int32, weight f32) per corner."""
    x = (grid[:, 0] + 1.0) * 0.5 * (W - 1)
    y = (grid[:, 1] + 1.0) * 0.5 * (W - 1)
    x0f, y0f = np.floor(x), np.floor(y)
    wx1, wy1 = (x - x0f).astype(np.float32), (y - y0f).astype(np.float32)
    wx0, wy0 = 1.0 - wx1, 1.0 - wy1
    x0 = np.clip(x0f, 0, W - 1).astype(np.int64)
    x1 = np.clip(x0f + 1, 0, W - 1).astype(np.int64)
    y0 = np.clip(y0f, 0, W - 1).astype(np.int64)
    y1 = np.clip(y0f + 1, 0, W - 1).astype(np.int64)
    return [
        (y0 * W + x0, wy0 * wx0),
        (y0 * W + x1, wy0 * wx1),
        (y1 * W + x0, wy1 * wx0),
        (y1 * W + x1, wy1 * wx1),
    ]


def _prep(inputs):
    """Host-side restructuring: sorting, padding, index tables, sparse-operator
    blocks. Returns (cfg, per_core_aux_list, post) where cfg fixes the (mesh-
    independent) instruction structure."""
    feats = [inputs["feat1"], inputs["feat2"], inputs["feat3"], inputs["feat4"]]
    av = np.asarray(inputs["aligned_verts"], np.float32)
    verts = np.asarray(inputs["verts_packed"], np.float32)
    enc = np.asarray(inputs["image_enc"], np.float32)
    edges = np.asarray(inputs["edges"], np.int64)

    for bn in ["bottleneck_b", "g0_b0", "g0_b1", "off_b"]:
        assert not np.any(np.asarray(inputs[bn])), f"{bn} nonzero: unsupported"
    assert not np.any(np.asarray(inputs["gb0"])) and not np.any(
        np.asarray(inputs["gb1"])
    ), "gb nonzero: unsupported"

    # per-mesh vertex sort (by finest-map cell) ----------------------------
    sigmas, invs, corners_all = [], [], []
    for m in range(B):
        grid = av[m, :, :2]
        cs = _corners(grid, MAPS[0][1])
        key = cs[0][0]  # y0*56+x0 of map 0
        sigma = np.argsort(key, kind="stable")
        inv = np.empty(V, np.int64)
        inv[sigma] = np.arange(V)
        sigmas.append(sigma)
        invs.append(inv)
        corners_all.append(
            [[(pix[sigma], w[sigma]) for (pix, w) in _corners(grid, Wm)]
             for (_, Wm) in MAPS]
        )

    # sampling schedule: per (map, vchunk) the union over meshes of touched
    # pixel tiles ---------------------------------------------------------
    ntile_map = [(Wm * Wm + 127) // 128 for (_, Wm) in MAPS]
    g_off = np.cumsum([0] + ntile_map)  # global G-tile offsets
    sched = []  # sched[mi][c] = list of pixel-tile indices
    for mi in range(4):
        per_c = []
        for c in range(NVCH):
            lo, hi = c * 512, min((c + 1) * 512, V)
            tiles = set()
            if lo < V:
                for m in range(B):
                    for (pix, _w) in corners_all[m][mi]:
                        pc = pix[lo:hi] // 128
                        tiles.update(np.unique(pc).tolist())
            per_c.append(sorted(tiles) if tiles else [0])
        per_c = [sorted(t) for t in per_c]
        np_m = max(len(t) for t in per_c)
        per_c = [t + [t[0]] * (np_m - len(t)) for t in per_c]  # pad (zero blocks)
        sched.append(per_c)
    np_list = [len(sched[mi][0]) for mi in range(4)]
    npair = sum(np_list) * NVCH

    # graph structure ------------------------------------------------------
    # directed edges sorted by dst, grouped per dst tile, padded to NSUB*128
    per_core = []
    for m in range(B):
        e = edges[m * E_PER:(m + 1) * E_PER] - m * V
        a = invs[m][e[:, 0]]
        b = invs[m][e[:, 1]]
        dst = np.concatenate([a, b])
        src = np.concatenate([b, a])
        order = np.lexsort((src, dst))
        dst, src = dst[order], src[order]
        tile_id = dst // 128
        counts = np.bincount(tile_id, minlength=NT)
        assert counts.max() <= NSUB * 128, f"NSUB too small: {counts.max()}"
        src_slots = np.zeros((NT, NSUB * 128), np.int32)
        dl_slots = np.full((NT, NSUB * 128), -1, np.int32)
        pos = 0
        for t in range(NT):
            cnt = counts[t]
            src_slots[t, :cnt] = src[pos:pos + cnt]
            dl_slots[t, :cnt] = dst[pos:pos + cnt] - t * 128
            pos += cnt
        # slot i of tile t -> subchunk j=i//128, partition p=i%128
        src_lin = src_slots.reshape(TOT_SUB * 128)
        # wrapped int16 for dma_gather: idx i at (i%16, i//16), replicated 8x
        srcw = np.tile(src_lin.reshape(-1, 16).T, (8, 1)).astype(np.int16)
        # dst_local per (partition, subchunk)
        dl = dl_slots.reshape(TOT_SUB, 128).T.copy().astype(np.int32)  # [128,TOT_SUB]

        # sampling blocks ---------------------------------------------------
        wsc = np.zeros((npair, 128, 512), np.float32)
        pi = 0
        for c in range(NVCH):
            lo, hi = c * 512, min((c + 1) * 512, V)
            for mi in range(4):
                seen = set()
                for t in sched[mi][c]:
                    blk = wsc[pi]
                    if lo < V and t not in seen:  # pad repeats stay zero
                        seen.add(t)
                        for (pix, w) in corners_all[m][mi]:
                            px = pix[lo:hi]
                            sel = (px >= t * 128) & (px < (t + 1) * 128)
                            jj = np.nonzero(sel)[0]
                            np.add.at(blk, (px[jj] - t * 128, jj), w[lo:hi][jj])
                    pi += 1
        assert pi == npair

        vt = np.zeros((3, VP), np.float32)
        vt[:, :V] = verts[m * V:(m + 1) * V][sigmas[m]].T

        aux = {
            "f1": np.ascontiguousarray(
                feats[0][m].reshape(256, -1)).astype(np.float32),
            "f2": np.ascontiguousarray(
                feats[1][m].reshape(512, -1)).astype(np.float32),
            "f3": np.ascontiguousarray(
                feats[2][m].reshape(1024, -1)).astype(np.float32),
            "f4": np.ascontiguousarray(
                feats[3][m].reshape(2048, -1)).astype(np.float32),
            "bw": np.ascontiguousarray(np.asarray(inputs["bottleneck_w"],
                                                  np.float32)),
            "wsc": wsc.reshape(npair * 128, 512),
            "srcw": np.ascontiguousarray(srcw),
            "dstloc": np.ascontiguousarray(dl),
            "iota": np.tile(np.arange(128, dtype=np.int32), (128, 1)),
            "vertsT": vt,
            "encc": enc[m].reshape(2, 128).T.copy(),  # [128, 2]
            "g0w0m": np.asarray(inputs["g0_w0"][:128], np.float32),
            "g0w0v": np.asarray(inputs["g0_w0"][128:131], np.float32),
            "g0w0e": np.ascontiguousarray(
                np.asarray(inputs["g0_w0"][131:387], np.float32)),
            "g0w1m": np.asarray(inputs["g0_w1"][:128], np.float32),
            "g0w1v": np.asarray(inputs["g0_w1"][128:131], np.float32),
            "g0w1e": np.ascontiguousarray(
                np.asarray(inputs["g0_w1"][131:387], np.float32)),
            "gw0": np.ascontiguousarray(
                np.asarray(inputs["gw0"], np.float32).transpose(1, 0, 2)
                .reshape(128, 7 * 128)),
            "gw1": np.ascontiguousarray(
                np.asarray(inputs["gw1"], np.float32).transpose(1, 0, 2)
                .reshape(128, 7 * 128)),
            "offw": np.asarray(inputs["off_w"], np.float32),
        }
        per_core.append(aux)

    cfg = {"sched": sched, "np_list": np_list, "npair": npair,
           "g_off": g_off.tolist(), "ntile_map": ntile_map}
    post = {"sigmas": sigmas}
    return cfg, per_core, post


def _build(cfg, shapes, dump=None, nlayers=8):
    """Build the SPMD Bass program (same instruction stream for all cores).

    dump: None | "x" — also emit the column-form activations [128, VP]
    after the last executed stage (sampling if nlayers==0).
    """
    nc = bacc.Bacc("TRN2", target_bir_lowering=False, debug=False, num_devices=B)
    ap = {}
    for name, arr in shapes.items():
        ap[name] = nc.dram_tensor(
            name, list(arr.shape), mybir.dt.from_np(arr.dtype),
            kind="ExternalInput").ap()
    out = nc.dram_tensor("out", [VP, 3], F32, kind="ExternalOutput").ap()
    xdump = (nc.dram_tensor("xdump", [128, VP], F32, kind="ExternalOutput").ap()
             if dump else None)
    h1d2 = [nc.dram_tensor("h1da", [VP, HID], F32).ap(),
            nc.dram_tensor("h1db", [VP, HID], F32).ap()]

    sched = cfg["sched"]
    np_list = cfg["np_list"]
    g_off = cfg["g_off"]
    ntile_map = cfg["ntile_map"]
    NGT = g_off[4]  # total G tiles

    with tile.TileContext(nc) as tc, ExitStack() as ctx:
        # ---------------- persistent pools ----------------
        pp = ctx.enter_context(tc.tile_pool(name="pers", bufs=1))
        xa = pp.tile([128, VP], F32, tag="xa")
        xb = pp.tile([128, VP], F32, tag="xb")
        srcw_t = pp.tile([128, TOT_SUB * 8], I16, tag="srcw")
        dstloc_t = pp.tile([128, TOT_SUB, 1], I32, tag="dstloc")
        iota_t = pp.tile([128, 1, 128], I32, tag="iota")
        w0_t = pp.tile([128, 7 * 128], F32, tag="w0")
        w1_t = pp.tile([128, 7 * 128], F32, tag="w1")
        g0_t = pp.tile([128, 6 * 128], F32, tag="g0")  # w0m,w1m,w0e(2),w1e(2)
        g0v_t = pp.tile([3, 256], F32, tag="g0v")      # w0v, w1v
        offw_t = pp.tile([128, 3], F32, tag="offw")
        ones_t = pp.tile([1, 128], F32, tag="ones")
        erow_t = pp.tile([1, 256], F32, tag="erow")    # e0row, e1row
        encc_t = pp.tile([128, 2], F32, tag="encc")

        nc.sync.dma_start(srcw_t[:], ap["srcw"][:])
        nc.sync.dma_start(
            dstloc_t[:], ap["dstloc"].rearrange("p (s o) -> p s o", o=1))
        nc.sync.dma_start(iota_t[:].rearrange("p o d -> p (o d)"),
                          ap["iota"][:])
        nc.sync.dma_start(w0_t[:], ap["gw0"][:])
        nc.sync.dma_start(w1_t[:], ap["gw1"][:])
        nc.sync.dma_start(g0_t[:, 0:128], ap["g0w0m"][:])
        nc.sync.dma_start(g0_t[:, 128:256], ap["g0w1m"][:])
        nc.sync.dma_start(
            g0_t[:, 256:512].rearrange("p (c h) -> p c h", h=128),
            ap["g0w0e"].rearrange("(c p) h -> p c h", p=128))
        nc.sync.dma_start(
            g0_t[:, 512:768].rearrange("p (c h) -> p c h", h=128),
            ap["g0w1e"].rearrange("(c p) h -> p c h", p=128))
        nc.sync.dma_start(g0v_t[:, 0:128], ap["g0w0v"][:])
        nc.sync.dma_start(g0v_t[:, 128:256], ap["g0w1v"][:])
        nc.sync.dma_start(offw_t[:], ap["offw"][:])
        nc.vector.memset(ones_t[:], 1.0)
        nc.sync.dma_start(encc_t[:], ap["encc"][:])

        pps = ctx.enter_context(tc.tile_pool(name="perspsum", bufs=2,
                                             space="PSUM"))

        # enc rank-1 rows: e{0,1} = g0_w{0,1}[131:387].T @ enc  -> [1,128]
        for k in range(2):
            pe = pps.tile([1, 128], F32, tag="pe")
            for cchunk in range(2):
                nc.tensor.matmul(
                    out=pe[:],
                    lhsT=encc_t[:, cchunk:cchunk + 1],
                    rhs=g0_t[:, 256 + k * 256 + cchunk * 128:
                             256 + k * 256 + cchunk * 128 + 128],
                    start=(cchunk == 0), stop=(cchunk == 1))
            nc.scalar.activation(erow_t[:, k * 128:(k + 1) * 128], pe[:],
                                 AF.Copy)

        # ---------------- phase 1: sampling ----------------
        with ExitStack() as sctx:
            sp = sctx.enter_context(tc.tile_pool(name="samp", bufs=1))
            spw = sctx.enter_context(tc.tile_pool(name="sampw", bufs=3))
            spp = sctx.enter_context(tc.tile_pool(name="samppsum", bufs=2,
                                                  space="PSUM"))
            spp1 = sctx.enter_context(tc.tile_pool(name="samppsum1", bufs=2,
                                                   space="PSUM"))
            bw_t = sp.tile([128, 30 * 128], F32, tag="bw")
            nc.sync.dma_start(
                bw_t[:].rearrange("p (c h) -> p c h", h=128),
                ap["bw"].rearrange("(c p) h -> p c h", p=128))
            g_sb = sp.tile([128, NGT * 128], F32, tag="gsb")

            for mi, (C, Wm) in enumerate(MAPS):
                HW = Wm * Wm
                ncc = C // 128
                fm_t = sp.tile([128, ncc * HW], F32, tag="fm")
                nc.sync.dma_start(
                    fm_t[:].rearrange("p (c hw) -> p c hw", c=ncc),
                    ap[f"f{mi+1}"].rearrange("(c p) hw -> p c hw", p=128))
                bwo = CH_OFF[mi] // 128
                for t in range(ntile_map[mi]):
                    p0 = t * 128
                    pcnt = min(128, HW - p0)
                    pg = spp.tile([128, 128], F32, tag="pg")
                    for cc in range(ncc):
                        nc.tensor.matmul(
                            out=pg[:pcnt, :],
                            lhsT=fm_t[:, cc * HW + p0: cc * HW + p0 + pcnt],
                            rhs=bw_t[:, (bwo + cc) * 128:(bwo + cc) * 128 + 128],
                            start=(cc == 0), stop=(cc == ncc - 1))
                    gt = g_off[mi] + t
                    nc.scalar.activation(
                        g_sb[:pcnt, gt * 128:gt * 128 + 128], pg[:pcnt, :],
                        AF.Copy)

            pair = 0
            for c in range(NVCH):
                ps = spp1.tile([128, 512], F32, tag="ps")
                pairs_c = []
                for mi in range(4):
                    for t in sched[mi][c]:
                        pairs_c.append((mi, t))
                for k, (mi, t) in enumerate(pairs_c):
                    HW = MAPS[mi][1] ** 2
                    pcnt = min(128, HW - t * 128)
                    wt = spw.tile([128, 512], F32, tag="wsc")
                    nc.sync.dma_start(
                        wt[:], ap["wsc"][pair * 128:(pair + 1) * 128, :])
                    gt = g_off[mi] + t
                    nc.tensor.matmul(
                        out=ps[:],
                        lhsT=g_sb[:pcnt, gt * 128:gt * 128 + 128],
                        rhs=wt[:pcnt, :],
                        start=(k == 0), stop=(k == len(pairs_c) - 1))
                    pair += 1
                nc.scalar.activation(xa[:, c * 512:(c + 1) * 512], ps[:],
                                     AF.Relu)

        # ---------------- phase 2: graph conv layers ----------------
        lp = ctx.enter_context(tc.tile_pool(name="lay", bufs=2))
        lph = ctx.enter_context(tc.tile_pool(name="layh", bufs=2))
        lpv = ctx.enter_context(tc.tile_pool(name="layv", bufs=2))
        psh = ctx.enter_context(tc.tile_pool(name="psumh", bufs=2, space="PSUM"))
        psx = ctx.enter_context(tc.tile_pool(name="psumx", bufs=2, space="PSUM"))

        HB = 7  # h1 write batch (tiles)
        cur, nxt = xa, xb
        for l in range(nlayers):
            h1d = h1d2[l % 2]
            h1_writes = []
            # h1 rows -> h1d (DRAM)
            for t0 in range(0, NT, HB):
                tb = min(HB, NT - t0)
                hst = lph.tile([128, HB * 128], F32, tag="hst")
                if l == 0:
                    vv = lpv.tile([3, HB * 128], F32, tag="vt")
                    nc.sync.dma_start(
                        vv[:, :tb * 128],
                        ap["vertsT"][:, t0 * 128:(t0 + tb) * 128])
                for ti in range(tb):
                    t = t0 + ti
                    ph = psh.tile([128, 128], F32, tag="ph")
                    if l == 0:
                        nc.tensor.matmul(
                            out=ph[:], lhsT=cur[:, t * 128:(t + 1) * 128],
                            rhs=g0_t[:, 128:256], start=True, stop=False)
                        vs = ti * 128
                        nc.tensor.matmul(
                            out=ph[:], lhsT=vv[:, vs:vs + 128],
                            rhs=g0v_t[:, 128:256], start=False, stop=False)
                        nc.tensor.matmul(
                            out=ph[:], lhsT=ones_t[:],
                            rhs=erow_t[:, 128:256], start=False, stop=True)
                    else:
                        nc.tensor.matmul(
                            out=ph[:], lhsT=cur[:, t * 128:(t + 1) * 128],
                            rhs=w1_t[:, (l - 1) * 128:l * 128],
                            start=True, stop=True)
                    nc.scalar.activation(hst[:, ti * 128:(ti + 1) * 128],
                                         ph[:], AF.Copy)
                h1_writes.append(nc.sync.dma_start(
                    h1d.rearrange("(n p) c -> p n c", p=128)[:, t0:t0 + tb, :],
                    hst[:, :tb * 128].rearrange("p (n c) -> p n c", c=128)))

            # gather groups + scatter matmuls
            for g in range(NGRP):
                msg = lp.tile([128, SUB_G, 128], F32, tag="msg")
                gi = nc.gpsimd.dma_gather(
                    out_ap=msg[:],
                    in_ap=h1d[:],
                    idxs_ap=srcw_t[:, g * SUB_G * 8:(g + 1) * SUB_G * 8],
                    num_idxs=SUB_G * 128,
                    num_idxs_reg=SUB_G * 128,
                    elem_size=HID,
                    single_packet=False,
                )
                for wi in h1_writes:
                    tile.add_dep_helper(gi.ins, wi.ins,
                                        reason="h1 RAW: gather after write")
                oh = lp.tile([128, SUB_G, 128], F32, tag="oh")
                nc.vector.tensor_tensor(
                    out=oh[:],
                    in0=dstloc_t[:, g * SUB_G:(g + 1) * SUB_G, :]
                    .to_broadcast([128, SUB_G, 128]),
                    in1=iota_t[:].to_broadcast([128, SUB_G, 128]),
                    op=mybir.AluOpType.is_equal)
                if l == 0:
                    vv2 = lpv.tile([3, GT * 128], F32, tag="vt2")
                    nc.sync.dma_start(
                        vv2[:],
                        ap["vertsT"][:, g * GT * 128:(g + 1) * GT * 128])
                for ti in range(GT):
                    t = g * GT + ti
                    px = psx.tile([128, 128], F32, tag="px")
                    if l == 0:
                        nc.tensor.matmul(
                            out=px[:], lhsT=g0_t[:, 0:128],
                            rhs=cur[:, t * 128:(t + 1) * 128],
                            start=True, stop=False)
                        nc.tensor.matmul(
                            out=px[:], lhsT=g0v_t[:, 0:128],
                            rhs=vv2[:, ti * 128:(ti + 1) * 128],
                            start=False, stop=False)
                        nc.tensor.matmul(
                            out=px[:], lhsT=erow_t[:, 0:128],
                            rhs=ones_t[:], start=False, stop=False)
                    else:
                        nc.tensor.matmul(
                            out=px[:], lhsT=w0_t[:, (l - 1) * 128:l * 128],
                            rhs=cur[:, t * 128:(t + 1) * 128],
                            start=True, stop=False)
                    for j in range(NSUB):
                        s = ti * NSUB + j
                        nc.tensor.matmul(
                            out=px[:], lhsT=msg[:, s, :], rhs=oh[:, s, :],
                            start=False, stop=(j == NSUB - 1))
                    nc.scalar.activation(nxt[:, t * 128:(t + 1) * 128], px[:],
                                         AF.Relu)
            cur, nxt = nxt, cur

        if xdump is not None:
            nc.sync.dma_start(xdump[:], cur[:])

        # ---------------- output ----------------
        OB = 12
        for t0 in range(0, NT, OB):
            tb = min(OB, NT - t0)
            ost = lph.tile([128, OB * 3], F32, tag="ost")
            for ti in range(tb):
                t = t0 + ti
                po = psh.tile([128, 3], F32, tag="po")
                nc.tensor.matmul(out=po[:], lhsT=cur[:, t * 128:(t + 1) * 128],
                                 rhs=offw_t[:], start=True, stop=True)
                nc.scalar.activation(ost[:, ti * 3:(ti + 1) * 3], po[:],
                                     AF.Copy)
            nc.sync.dma_start(
                out.rearrange("(n p) c -> p n c", p=128)[:, t0:t0 + tb, :],
                ost[:, :tb * 3].rearrange("p (n c) -> p n c", c=3))

    nc.compile()
    return nc


_CACHE = {}


def kernel(**inputs) -> np.ndarray:
    cfg, per_core, post = _prep(inputs)
    key = (cfg["npair"], tuple(cfg["np_list"]))
    if key not in _CACHE:
        _CACHE[key] = _build(cfg, per_core[0])
    nc = _CACHE[key]
    res = run_bass_kernel_spmd(nc, per_core, list(range(B)))
    outs = np.empty((B, V, 3), np.float32)
    for m in range(B):
        rows = res.results[m]["out"][:V]
        outs[m][post["sigmas"][m]] = rows
    return outs.reshape(B * V, 3)


if __name__ == "__main__":
    rng = np.random.default_rng(0)
    pass
